# revision 6
# baseline (speedup 1.0000x reference)
"""CRF mean-NLL kernel for Trainium2 (8 NeuronCores).

Problem: B=1024 sequences of length S=1024 with T=16 tags.
  nll = mean_b( logZ_b - gold_b )

Device strategy (SPMD, one uniform Bass/Tile program on 8 cores):
  - Sequence split 2-way: cores 0-3 run the FORWARD half (s in [0,512)),
    cores 4-7 run the BACKWARD half (s in [512,1024)); they meet at the
    midpoint and the (tiny) combine is a per-b dot product done on host.
  - Batch split 4-way: core c handles b-quarter q = c % 4 (256 rows).
  - Linear-domain recursion with the tag dimension on SBUF partitions,
    packed 8 groups x 16 tags = 128 partitions, 32 batch columns free:
        state <- (E8^T state) * u_t          (PE matmul + DVE multiply)
    where E8 = blockdiag(exp(transitions)) and u_t = exp(em_t - kappa).
    kappa = log(16) + 0.5 keeps magnitudes O(1) (deterministic log-shift,
    re-added on host), so no per-step renormalization is needed.
  - Both roles run the identical program: the role-specific init state is
    pre-solved on host in f64 (fwd: E^-T exp(start), bwd: E^-1 exp(end))
    so step 0 can go through the same matmul as every other step.
  - Gold emission score sum_s em[b,s,tag[b,s]] is computed on device from
    a second, natural-layout (batch-on-partitions) copy of emissions via
    a one-hot build (gpsimd) + multiply-accumulate (scalar_tensor_tensor).
  - The remaining gold terms (transition pairs, start/end) depend only on
    tags + the tiny parameter tables and are summed on host.
"""

import os
import sys

import numpy as np

for _p in ("/opt/trn_rl_repo",):
    if os.path.isdir(_p) and _p not in sys.path:
        sys.path.insert(0, _p)

B, S, T = 1024, 1024, 16
NCORES = 8
G = 8                 # tag-groups packed on partitions
BB = 32               # batch columns per group (8*32 = 256 b per core)
BQ = G * BB           # 256 batch rows per core
SH = S // 2           # 512 steps per core
CHUNK_STEPS = 64      # u-chunk = 64 steps -> [128, 2048] tiles
NCHUNK = SH // CHUNK_STEPS
KAPPA = float(np.log(16.0) + 0.5)

_PROGRAM = None
LAST_RESULTS = None   # BassKernelResults of the most recent run (for test.py)


def _build_program(trace_ready=False):
    """Build the uniform SPMD Bass program (compiled once, cached)."""
    global _PROGRAM
    if _PROGRAM is not None:
        return _PROGRAM

    import concourse.bacc as bacc
    import concourse.tile as tile
    from concourse import mybir

    f32 = mybir.dt.float32
    bf16 = mybir.dt.bfloat16
    Alu = mybir.AluOpType
    Act = mybir.ActivationFunctionType

    nc = bacc.Bacc(
        "TRN2",
        target_bir_lowering=False,
        debug=False,
        enable_asserts=False,
        num_devices=NCORES,
    )

    emlin = nc.dram_tensor("emlin", [128, SH * BB], f32, kind="ExternalInput").ap()
    emnat = nc.dram_tensor("emnat", [2, 128, SH * T], bf16, kind="ExternalInput").ap()
    tagsn = nc.dram_tensor("tagsn", [2, 128, SH], bf16, kind="ExternalInput").ap()
    iota16 = nc.dram_tensor("iota16", [128, T], bf16, kind="ExternalInput").ap()
    e8 = nc.dram_tensor("e8", [128, 128], f32, kind="ExternalInput").ap()
    init = nc.dram_tensor("init", [128, BB], f32, kind="ExternalInput").ap()
    kbias = nc.dram_tensor("kbias", [128, 1], f32, kind="ExternalInput").ap()

    state_out = nc.dram_tensor("state", [128, BB], f32, kind="ExternalOutput").ap()
    goldem_out = nc.dram_tensor("goldem", [128, 2], f32, kind="ExternalOutput").ap()

    with tile.TileContext(nc) as tc:
        with (
            tc.tile_pool(name="const", bufs=1) as constp,
            tc.tile_pool(name="emchunk", bufs=3) as emp,
            tc.tile_pool(name="u", bufs=NCHUNK) as up,
            tc.tile_pool(name="state", bufs=3) as sp,
            tc.tile_pool(name="psum", bufs=4, space="PSUM") as pp,
            tc.tile_pool(name="nat", bufs=2) as natp,
            tc.tile_pool(name="gold", bufs=2) as gp,
        ):
            e8_sb = constp.tile([128, 128], f32)
            nc.sync.dma_start(e8_sb[:], e8[:])
            iota_sb = constp.tile([128, T], bf16)
            nc.sync.dma_start(iota_sb[:], iota16[:])
            kb_sb = constp.tile([128, 1], f32)
            nc.sync.dma_start(kb_sb[:], kbias[:])

            state = sp.tile([128, BB], f32)
            nc.sync.dma_start(state[:], init[:])

            # bulk u = exp(em - kappa), chunked so the chain can start early
            cw = CHUNK_STEPS * BB
            u_tiles = []
            for k in range(NCHUNK):
                emc = emp.tile([128, cw], f32, tag="emc")
                nc.sync.dma_start(emc[:], emlin[:, k * cw:(k + 1) * cw])
                u_k = up.tile([128, cw], f32, tag="u")
                nc.scalar.activation(u_k[:], emc[:], Act.Exp, bias=kb_sb[:])
                u_tiles.append(u_k)

            # the 512-step recursion
            for t in range(SH):
                ps = pp.tile([128, BB], f32, tag="ps")
                nc.tensor.matmul(ps[:], e8_sb[:], state[:], start=True, stop=True)
                new_state = sp.tile([128, BB], f32, tag="state")
                u_k = u_tiles[t // CHUNK_STEPS]
                off = (t % CHUNK_STEPS) * BB
                nc.vector.tensor_tensor(
                    new_state[:], ps[:], u_k[:, off:off + BB], op=Alu.mult
                )
                state = new_state
            nc.sync.dma_start(state_out[:], state[:])

            # gold emission gather: one-hot(tag) * em, summed over free dim
            for k in range(2):
                en = natp.tile([128, SH * T], bf16, tag="en")
                nc.sync.dma_start(en[:], emnat[k])
                tg = natp.tile([128, SH], bf16, tag="tg")
                nc.sync.dma_start(tg[:], tagsn[k])

                oh = gp.tile([128, SH * T], bf16, tag="oh")
                oh3 = oh[:].rearrange("p (s j) -> p s j", j=T)
                tg3 = tg[:].unsqueeze(2).broadcast_to([128, SH, T])
                io3 = iota_sb[:].unsqueeze(1).broadcast_to([128, SH, T])
                nc.vector.tensor_tensor(oh3, tg3, io3, op=Alu.is_equal)

                scrap = gp.tile([128, SH * T], bf16, tag="scrap")
                gacc = gp.tile([128, 1], f32, tag="gacc")
                nc.vector.scalar_tensor_tensor(
                    scrap[:], en[:], 0.0, oh[:],
                    op0=Alu.bypass, op1=Alu.mult, accum_out=gacc[:],
                )
                nc.sync.dma_start(goldem_out[:, k:k + 1], gacc[:])

    nc.compile()
    _PROGRAM = nc
    return nc


def _host_prep(emissions, tags, transitions, start_transitions, end_transitions):
    """Build the 8 per-core input dicts."""
    import ml_dtypes

    em = np.ascontiguousarray(emissions, dtype=np.float32)
    tg = np.asarray(tags)
    Tm = np.asarray(transitions, dtype=np.float64)
    E = np.exp(Tm)                       # E[i,j] = exp(trans[i,j])
    sv = np.exp(np.asarray(start_transitions, dtype=np.float64))
    ev = np.exp(np.asarray(end_transitions, dtype=np.float64))

    init_f = np.linalg.solve(E.T, sv).astype(np.float32)   # E^-T exp(start)
    init_b = np.linalg.solve(E, ev).astype(np.float32)     # E^-1 exp(end)

    e8_f = np.zeros((128, 128), np.float32)
    e8_b = np.zeros((128, 128), np.float32)
    Ef32 = E.astype(np.float32)
    for g in range(G):
        e8_f[g * T:(g + 1) * T, g * T:(g + 1) * T] = Ef32
        e8_b[g * T:(g + 1) * T, g * T:(g + 1) * T] = Ef32.T

    iota = np.broadcast_to(
        np.arange(T, dtype=np.float32), (128, T)
    ).astype(ml_dtypes.bfloat16)

    in_maps = []
    for c in range(NCORES):
        fwd = c < 4
        q = c % 4
        emq = em[q * BQ:(q + 1) * BQ]                      # [256, 1024, 16]
        half = emq[:, :SH] if fwd else emq[:, SH:]         # [256, 512, 16]

        # chain layout [g, j, tau, bb]; bwd walks time reversed
        hh = half if fwd else half[:, ::-1]
        emlin = (
            hh.reshape(G, BB, SH, T)
            .transpose(0, 3, 2, 1)
            .reshape(128, SH * BB)
        )
        emlin = np.ascontiguousarray(emlin, dtype=np.float32)

        # natural layout for the gold gather (not time-reversed)
        emnat = half.reshape(2, 128, SH * T).astype(ml_dtypes.bfloat16)
        tgq = tg[q * BQ:(q + 1) * BQ, : SH] if fwd else tg[q * BQ:(q + 1) * BQ, SH:]
        tagsn = tgq.reshape(2, 128, SH).astype(np.float32).astype(ml_dtypes.bfloat16)

        initv = init_f if fwd else init_b                  # [16]
        init_tile = np.broadcast_to(
            np.tile(initv, G)[:, None], (128, BB)
        ).astype(np.float32)
        init_tile = np.ascontiguousarray(init_tile)

        in_maps.append({
            "emlin": emlin,
            "emnat": np.ascontiguousarray(emnat),
            "tagsn": np.ascontiguousarray(tagsn),
            "iota16": iota,
            "e8": e8_f if fwd else e8_b,
            "init": init_tile,
            "kbias": np.full((128, 1), -KAPPA, np.float32),
        })
    return in_maps, E


def _reference_numpy(emissions, tags, mask, transitions,
                     start_transitions, end_transitions):
    """Exact numpy replica of reference.py (fallback for unexpected inputs)."""
    em = np.asarray(emissions, dtype=np.float64)
    tg = np.asarray(tags).astype(np.int64)
    mk = np.asarray(mask).astype(bool)
    Tm = np.asarray(transitions, dtype=np.float64)
    sv = np.asarray(start_transitions, dtype=np.float64)
    ev = np.asarray(end_transitions, dtype=np.float64)
    Bn, Sn, Tn = em.shape

    bidx = np.arange(Bn)
    score = sv[tg[:, 0]] + em[bidx, 0, tg[:, 0]]
    emit = np.take_along_axis(em, tg[:, :, None], axis=2)[:, :, 0]
    trans = Tm[tg[:, 1:], tg[:, :-1]]
    m = mk[:, 1:].astype(np.float64)
    gold = score + np.sum((emit[:, 1:] + trans) * m, axis=1)
    last_idx = mk.astype(np.int64).sum(1) - 1
    last_tags = np.take_along_axis(tg, last_idx[:, None], axis=1)[:, 0]
    gold = gold + ev[last_tags]

    sc = sv[None, :] + em[:, 0]
    for t in range(1, Sn):
        nxt = sc[:, :, None] + Tm[None, :, :] + em[:, t][:, None, :]
        mx = nxt.max(axis=1)
        nxt = np.log(np.exp(nxt - mx[:, None, :]).sum(axis=1)) + mx
        sc = np.where(mk[:, t][:, None], nxt, sc)
    sc = sc + ev[None, :]
    mx = sc.max(axis=1)
    logZ = np.log(np.exp(sc - mx[:, None]).sum(axis=1)) + mx
    return np.float32(np.mean(logZ - gold))


def kernel(emissions, tags, mask, transitions, start_transitions,
           end_transitions):
    global LAST_RESULTS
    emissions = np.asarray(emissions)
    tags = np.asarray(tags)
    mask = np.asarray(mask)
    transitions = np.asarray(transitions)
    start_transitions = np.asarray(start_transitions)
    end_transitions = np.asarray(end_transitions)

    if (emissions.shape != (B, S, T)) or not bool(np.all(mask)):
        return _reference_numpy(emissions, tags, mask, transitions,
                                start_transitions, end_transitions)

    from concourse.bass_utils import run_bass_kernel_spmd

    nc = _build_program()
    in_maps, E = _host_prep(emissions, tags, transitions,
                            start_transitions, end_transitions)

    trace = os.environ.get("CRF_TRACE", "0") == "1"
    res = run_bass_kernel_spmd(nc, in_maps, list(range(NCORES)), trace=trace)
    LAST_RESULTS = res

    # ---- host combine (tiny) ----
    tg = tags.astype(np.int64)
    Tm = np.asarray(transitions, dtype=np.float64)
    sv = np.asarray(start_transitions, dtype=np.float64)
    ev = np.asarray(end_transitions, dtype=np.float64)

    logZ = np.empty(B, np.float64)
    gold_em = np.empty(B, np.float64)
    for q in range(4):
        a = res.results[q]["state"].astype(np.float64).reshape(G, T, BB)
        sbk = res.results[q + 4]["state"].astype(np.float64).reshape(G, T, BB)
        bvec = np.einsum("ij,gjb->gib", E, sbk)        # E @ s = beta_511
        z = np.einsum("gib,gib->gb", a, bvec)          # [G, BB]
        logZ[q * BQ:(q + 1) * BQ] = (
            np.log(z) + (2 * SH) * KAPPA
        ).reshape(BQ)                                  # b = g*32+bb order

        ge = (res.results[q]["goldem"].astype(np.float64)
              + res.results[q + 4]["goldem"].astype(np.float64))  # [128, 2]
        gold_em[q * BQ:(q + 1) * BQ] = ge.T.reshape(BQ)  # b = k*128 + p order

    gold = (
        gold_em
        + sv[tg[:, 0]]
        + ev[tg[:, -1]]
        + Tm[tg[:, 1:], tg[:, :-1]].sum(axis=1)
    )
    return np.float32(np.mean(logZ - gold))


# revision 13
# speedup vs baseline: 7659.2695x; 7659.2695x over previous
"""CRF mean-NLL kernel for Trainium2 (8 NeuronCores).

Problem: B=1024 sequences of length S=1024 with T=16 tags.
  nll = mean_b( logZ_b - gold_b )

Device strategy (SPMD, one uniform Bass/Tile program on 8 cores):
  - Sequence split 2-way: cores 0-3 run the FORWARD half (s in [0,512)),
    cores 4-7 run the BACKWARD half (s in [512,1024)); they meet at the
    midpoint and the (tiny) combine is a per-b dot product done on host.
  - Batch split 4-way: core c handles b-quarter q = c % 4 (256 rows).
  - Linear-domain recursion with the tag dimension on SBUF partitions,
    packed 8 groups x 16 tags = 128 partitions, 32 batch columns free:
        state <- (E8^T state) * u_t          (PE matmul + DVE multiply)
    where E8 = blockdiag(exp(transitions)) and u_t = exp(em_t - kappa).
    kappa = log(16) + 0.5 keeps magnitudes O(1) (deterministic log-shift,
    re-added on host), so no per-step renormalization is needed.
  - Both roles run the identical program: the role-specific init state is
    pre-solved on host in f64 (fwd: E^-T exp(start), bwd: E^-1 exp(end))
    so step 0 can go through the same matmul as every other step.
  - Gold emission score sum_s em[b,s,tag[b,s]] is computed on device from
    a second, natural-layout (batch-on-partitions) copy of emissions via
    a one-hot build (gpsimd) + multiply-accumulate (scalar_tensor_tensor).
  - The remaining gold terms (transition pairs, start/end) depend only on
    tags + the tiny parameter tables and are summed on host.
"""

import os
import sys

import numpy as np

for _p in ("/opt/trn_rl_repo",):
    if os.path.isdir(_p) and _p not in sys.path:
        sys.path.insert(0, _p)

B, S, T = 1024, 1024, 16
NCORES = 8
G = 8                 # tag-groups packed on partitions
BB = 32               # batch columns per group (8*32 = 256 b per core)
BQ = G * BB           # 256 batch rows per core
SH = S // 2           # 512 steps per core
CHUNK_STEPS = 64      # u-chunk = 64 steps -> [128, 2048] tiles
NCHUNK = SH // CHUNK_STEPS
KAPPA = float(np.log(16.0) + 0.5)

_PROGRAM = None
LAST_RESULTS = None   # BassKernelResults of the most recent run (for test.py)


def _build_program(trace_ready=False):
    """Build the uniform SPMD Bass program (compiled once, cached)."""
    global _PROGRAM
    if _PROGRAM is not None:
        return _PROGRAM

    import concourse.bacc as bacc
    import concourse.tile as tile
    from concourse import mybir

    f32 = mybir.dt.float32
    bf16 = mybir.dt.bfloat16
    Alu = mybir.AluOpType
    Act = mybir.ActivationFunctionType

    nc = bacc.Bacc(
        "TRN2",
        target_bir_lowering=False,
        debug=False,
        enable_asserts=False,
        num_devices=NCORES,
    )

    emlin = nc.dram_tensor("emlin", [128, SH * BB], f32, kind="ExternalInput").ap()
    emnat = nc.dram_tensor("emnat", [2, 128, SH * T], bf16, kind="ExternalInput").ap()
    tagsn = nc.dram_tensor("tagsn", [2, 128, SH], bf16, kind="ExternalInput").ap()
    iota16 = nc.dram_tensor("iota16", [128, T], bf16, kind="ExternalInput").ap()
    e8 = nc.dram_tensor("e8", [128, 128], bf16, kind="ExternalInput").ap()
    initv = nc.dram_tensor("initv", [128, 1], f32, kind="ExternalInput").ap()
    kbias = nc.dram_tensor("kbias", [128, 1], f32, kind="ExternalInput").ap()

    state_out = nc.dram_tensor("state", [128, BB], f32, kind="ExternalOutput").ap()
    goldem_out = nc.dram_tensor("goldem", [128, 2], f32, kind="ExternalOutput").ap()

    with tile.TileContext(nc) as tc:
        with (
            tc.tile_pool(name="const", bufs=1) as constp,
            tc.tile_pool(name="emchunk", bufs=3) as emp,
            tc.tile_pool(name="u", bufs=NCHUNK) as up,
            tc.tile_pool(name="state", bufs=3) as sp,
            tc.tile_pool(name="psum", bufs=4, space="PSUM") as pp,
            tc.tile_pool(name="nat", bufs=2) as natp,
            tc.tile_pool(name="gold", bufs=2) as gp,
        ):
            e8_sb = constp.tile([128, 128], bf16)
            nc.sync.dma_start(e8_sb[:], e8[:])
            iota_sb = constp.tile([128, T], bf16)
            nc.sync.dma_start(iota_sb[:], iota16[:])
            kb_sb = constp.tile([128, 1], f32)
            nc.sync.dma_start(kb_sb[:], kbias[:])
            iv_sb = constp.tile([128, 1], f32)
            nc.sync.dma_start(iv_sb[:], initv[:])

            # bulk u = exp(em - kappa), chunked so the chain can start early
            cw = CHUNK_STEPS * BB
            u_tiles = []
            for k in range(NCHUNK):
                emc = emp.tile([128, cw], f32, tag="emc")
                nc.sync.dma_start(emc[:], emlin[:, k * cw:(k + 1) * cw])
                u_k = up.tile([128, cw], f32, tag="u")
                nc.scalar.activation(u_k[:], emc[:], Act.Exp, bias=kb_sb[:])
                u_tiles.append(u_k)

            # step 0: state = u_0 * initv  (fwd: exp(start), bwd: exp(end))
            state = sp.tile([128, BB], bf16, tag="state")
            nc.vector.tensor_scalar_mul(state[:], u_tiles[0][:, 0:BB], iv_sb[:])

            # steps 1..511 of the recursion
            for t in range(1, SH):
                ps = pp.tile([128, BB], f32, tag="ps")
                nc.tensor.matmul(ps[:], e8_sb[:], state[:], start=True, stop=True)
                last = t == SH - 1
                new_state = sp.tile([128, BB], f32 if last else bf16, tag="state")
                u_k = u_tiles[t // CHUNK_STEPS]
                off = (t % CHUNK_STEPS) * BB
                nc.vector.tensor_tensor(
                    new_state[:], ps[:], u_k[:, off:off + BB], op=Alu.mult
                )
                state = new_state
            nc.sync.dma_start(state_out[:], state[:])

            # gold emission gather: one-hot(tag) * em, summed over free dim
            for k in range(2):
                en = natp.tile([128, SH * T], bf16, tag="en")
                nc.sync.dma_start(en[:], emnat[k])
                tg = natp.tile([128, SH], bf16, tag="tg")
                nc.sync.dma_start(tg[:], tagsn[k])

                oh = gp.tile([128, SH * T], bf16, tag="oh")
                oh3 = oh[:].rearrange("p (s j) -> p s j", j=T)
                tg3 = tg[:].unsqueeze(2).broadcast_to([128, SH, T])
                io3 = iota_sb[:].unsqueeze(1).broadcast_to([128, SH, T])
                nc.vector.tensor_tensor(oh3, tg3, io3, op=Alu.is_equal)

                scrap = gp.tile([128, SH * T], bf16, tag="scrap")
                gacc = gp.tile([128, 1], f32, tag="gacc")
                nc.vector.scalar_tensor_tensor(
                    scrap[:], en[:], 0.0, oh[:],
                    op0=Alu.bypass, op1=Alu.mult, accum_out=gacc[:],
                )
                nc.sync.dma_start(goldem_out[:, k:k + 1], gacc[:])

    nc.compile()
    _PROGRAM = nc
    return nc


def _host_prep(emissions, tags, transitions, start_transitions, end_transitions):
    """Build the 8 per-core input dicts."""
    import ml_dtypes

    em = np.ascontiguousarray(emissions, dtype=np.float32)
    tg = np.asarray(tags)
    Tm = np.asarray(transitions, dtype=np.float64)
    E = np.exp(Tm)                       # E[i,j] = exp(trans[i,j])
    sv = np.exp(np.asarray(start_transitions, dtype=np.float64))
    ev = np.exp(np.asarray(end_transitions, dtype=np.float64))

    e8_f = np.zeros((128, 128), np.float32)
    e8_b = np.zeros((128, 128), np.float32)
    Ef32 = E.astype(np.float32)
    for g in range(G):
        e8_f[g * T:(g + 1) * T, g * T:(g + 1) * T] = Ef32
        e8_b[g * T:(g + 1) * T, g * T:(g + 1) * T] = Ef32.T
    e8_f = e8_f.astype(ml_dtypes.bfloat16)
    e8_b = e8_b.astype(ml_dtypes.bfloat16)

    iota = np.broadcast_to(
        np.arange(T, dtype=np.float32), (128, T)
    ).astype(ml_dtypes.bfloat16)

    in_maps = []
    for c in range(NCORES):
        fwd = c < 4
        q = c % 4
        emq = em[q * BQ:(q + 1) * BQ]                      # [256, 1024, 16]
        half = emq[:, :SH] if fwd else emq[:, SH:]         # [256, 512, 16]

        # chain layout [g, j, tau, bb]; bwd walks time reversed
        hh = half if fwd else half[:, ::-1]
        emlin = (
            hh.reshape(G, BB, SH, T)
            .transpose(0, 3, 2, 1)
            .reshape(128, SH * BB)
        )
        emlin = np.ascontiguousarray(emlin, dtype=np.float32)

        # natural layout for the gold gather (not time-reversed)
        emnat = half.reshape(2, 128, SH * T).astype(ml_dtypes.bfloat16)
        tgq = tg[q * BQ:(q + 1) * BQ, : SH] if fwd else tg[q * BQ:(q + 1) * BQ, SH:]
        tagsn = tgq.reshape(2, 128, SH).astype(np.float32).astype(ml_dtypes.bfloat16)

        iv = sv if fwd else ev                             # [16]
        initv = np.ascontiguousarray(
            np.tile(iv, G)[:, None], dtype=np.float32
        )

        in_maps.append({
            "emlin": emlin,
            "emnat": np.ascontiguousarray(emnat),
            "tagsn": np.ascontiguousarray(tagsn),
            "iota16": iota,
            "e8": e8_f if fwd else e8_b,
            "initv": initv,
            "kbias": np.full((128, 1), -KAPPA, np.float32),
        })
    return in_maps, E


def _reference_numpy(emissions, tags, mask, transitions,
                     start_transitions, end_transitions):
    """Exact numpy replica of reference.py (fallback for unexpected inputs)."""
    em = np.asarray(emissions, dtype=np.float64)
    tg = np.asarray(tags).astype(np.int64)
    mk = np.asarray(mask).astype(bool)
    Tm = np.asarray(transitions, dtype=np.float64)
    sv = np.asarray(start_transitions, dtype=np.float64)
    ev = np.asarray(end_transitions, dtype=np.float64)
    Bn, Sn, Tn = em.shape

    bidx = np.arange(Bn)
    score = sv[tg[:, 0]] + em[bidx, 0, tg[:, 0]]
    emit = np.take_along_axis(em, tg[:, :, None], axis=2)[:, :, 0]
    trans = Tm[tg[:, 1:], tg[:, :-1]]
    m = mk[:, 1:].astype(np.float64)
    gold = score + np.sum((emit[:, 1:] + trans) * m, axis=1)
    last_idx = mk.astype(np.int64).sum(1) - 1
    last_tags = np.take_along_axis(tg, last_idx[:, None], axis=1)[:, 0]
    gold = gold + ev[last_tags]

    sc = sv[None, :] + em[:, 0]
    for t in range(1, Sn):
        nxt = sc[:, :, None] + Tm[None, :, :] + em[:, t][:, None, :]
        mx = nxt.max(axis=1)
        nxt = np.log(np.exp(nxt - mx[:, None, :]).sum(axis=1)) + mx
        sc = np.where(mk[:, t][:, None], nxt, sc)
    sc = sc + ev[None, :]
    mx = sc.max(axis=1)
    logZ = np.log(np.exp(sc - mx[:, None]).sum(axis=1)) + mx
    return np.float32(np.mean(logZ - gold))


def _ensure_ntff_hook():
    """Register the axon NTFF profile hook if the image lacks antenv.axon_hooks."""
    try:
        from antenv.axon_hooks import get_axon_ntff_profile_hook  # noqa: F401
        return
    except ImportError:
        pass
    import types
    try:
        import antenv
    except ImportError:
        antenv = types.ModuleType("antenv")
        sys.modules["antenv"] = antenv
    from trn_agent_boot.trn_boot import _ntff_profile_via_ctypes
    mod = types.ModuleType("antenv.axon_hooks")
    _state = {"h": None}
    mod.set_axon_ntff_profile_hook = lambda h: _state.__setitem__("h", h)
    mod.get_axon_ntff_profile_hook = lambda: _state["h"]
    sys.modules["antenv.axon_hooks"] = mod
    antenv.axon_hooks = mod
    h = _ntff_profile_via_ctypes("/opt/axon/libaxon_pjrt.so")
    if h is not None:
        mod.set_axon_ntff_profile_hook(h)


def kernel(emissions, tags, mask, transitions, start_transitions,
           end_transitions):
    global LAST_RESULTS
    emissions = np.asarray(emissions)
    tags = np.asarray(tags)
    mask = np.asarray(mask)
    transitions = np.asarray(transitions)
    start_transitions = np.asarray(start_transitions)
    end_transitions = np.asarray(end_transitions)

    if (emissions.shape != (B, S, T)) or not bool(np.all(mask)):
        return _reference_numpy(emissions, tags, mask, transitions,
                                start_transitions, end_transitions)

    import concourse.bass_utils as bass_utils
    from concourse.bass_utils import run_bass_kernel_spmd

    nc = _build_program()
    in_maps, E = _host_prep(emissions, tags, transitions,
                            start_transitions, end_transitions)

    trace = os.environ.get("CRF_TRACE", "0") == "1"
    kw = {}
    if trace:
        _ensure_ntff_hook()
        bass_utils.upload_artifacts = lambda d: f"local:{d}"
        kw["tmpdir"] = os.environ.get("CRF_TRACE_DIR") or None
    res = run_bass_kernel_spmd(nc, in_maps, list(range(NCORES)), trace=trace, **kw)
    LAST_RESULTS = res

    # ---- host combine (tiny) ----
    tg = tags.astype(np.int64)
    Tm = np.asarray(transitions, dtype=np.float64)
    sv = np.asarray(start_transitions, dtype=np.float64)
    ev = np.asarray(end_transitions, dtype=np.float64)

    logZ = np.empty(B, np.float64)
    gold_em = np.empty(B, np.float64)
    for q in range(4):
        a = res.results[q]["state"].astype(np.float64).reshape(G, T, BB)
        sbk = res.results[q + 4]["state"].astype(np.float64).reshape(G, T, BB)
        bvec = np.einsum("ij,gjb->gib", E, sbk)        # E @ s = beta_511
        z = np.einsum("gib,gib->gb", a, bvec)          # [G, BB]
        logZ[q * BQ:(q + 1) * BQ] = (
            np.log(z) + (2 * SH) * KAPPA
        ).reshape(BQ)                                  # b = g*32+bb order

        ge = (res.results[q]["goldem"].astype(np.float64)
              + res.results[q + 4]["goldem"].astype(np.float64))  # [128, 2]
        gold_em[q * BQ:(q + 1) * BQ] = ge.T.reshape(BQ)  # b = k*128 + p order

    gold = (
        gold_em
        + sv[tg[:, 0]]
        + ev[tg[:, -1]]
        + Tm[tg[:, 1:], tg[:, :-1]].sum(axis=1)
    )
    return np.float32(np.mean(logZ - gold))
